# revision 4
# baseline (speedup 1.0000x reference)
"""Trainium2 Bass kernel for GQA multi-head attention (RoPE + padding|causal mask).

Sequence-sharded, collective-free: 8 cores = 2 (batch) x 4 (query windows of
512 rows). Each core computes K/V for the full sequence, Q for its own window,
attention for all 16 q heads over its window, and its own [512, 1024] slice of
the output projection. No inter-core communication.

v2 redesign vs v1:
  * Per-core KEY-TILE PERMUTATION (host side): each core's 4 diagonal
    key tiles (the ones overlapping its query window) are moved to slots
    0-3; attention is permutation-invariant over keys. The explicit
    pad|causal mask multiply then only runs on slots 0-3. Slots 4-15 are
    either all-visible (tiles before the window: causal always holds) or
    pad-only (tiles after the window: causal never holds); the pad-only
    mask is folded into the exp as a per-partition bias of -30000 on
    padded keys (exp -> 0), which costs nothing (bias is already an
    activation operand).
  * Q-window inputs (xq/cosQ/sinQ) dropped: after permutation the window
    is always tiles 0-3, so Q reads fixed slices of xT/cosA.
  * RoPE partner swap via DVE stream_shuffle (partners laid out +-16
    partitions apart, inside one 32-row quadrant) instead of 4 SBUF->SBUF
    DMAs per rope block.
  * Softmax normalize reads AV PSUM directly (no staging copies); the two
    denominator rows are copied to SBUF by ScalarE, one partition
    broadcast + one reciprocal serve both heads.
  * All [128, C] bf16 inputs consolidated into one "blob" tensor, [1, C]
    vectors into "smalls", plus a tiny fp32 "ebias": 3 input args total
    (per-arg per-call dispatch cost is ~20us under the PJRT path).
  * Output in bf16 (halves per-call output staging).
"""

import sys

if "/opt/trn_rl_repo" not in sys.path:
    sys.path.insert(0, "/opt/trn_rl_repo")

import numpy as np
import ml_dtypes

BF_NP = ml_dtypes.bfloat16

import concourse.mybir as mybir
import concourse.tile as tile
from concourse import bacc
from concourse.bass_utils import run_bass_kernel_spmd

B, S, D = 2, 2048, 1024
H_Q, H_KV, DK, DV = 16, 8, 64, 64
N_CORES = 8
P = 128
W = 512          # query window per core
FP = mybir.dt.float32
BF = mybir.dt.bfloat16
SCALE = 1.0 / 8.0  # 1/sqrt(DK)
NK = D // P      # 8 k-tiles over the model dim
NSK = S // P     # 16 sk tiles
VC = DV + 1      # 65 = V cols + ones col per head
VROW = H_KV * VC  # 520 cols of vv per sk tile
NEG = -30000.0   # "minus infinity" for the pad bias (exp -> 0)

# stream_shuffle mask: rotate each 32-partition quadrant by 16 (RoPE partner)
SWAP16 = list(range(16, 32)) + list(range(16))

# ---- blob column layout (bf16, [128, BLOB_C]) ----
_off = 0
def _seg(n):
    global _off
    s = slice(_off, _off + n)
    _off += n
    return s
SL_XT = _seg(NK * S)            # 16384
SL_WK = _seg(NK * H_KV * DK)    # 4096
SL_WQ = _seg(NK * H_Q * DK)     # 8192
SL_WV = _seg(NK * VROW)         # 4160
SL_COS = _seg(S)                # 2048
SL_SIN = _seg(S)                # 2048
SL_MSK = _seg(4 * W)            # 2048
SL_WO = _seg(NK * D)            # 8192
# ebias block (bf16 in the blob, converted to fp32 on device):
# exp pad-bias per slot, then bk per kT tile, bq per qT tile
EB_PAD = slice(0, NSK)
EB_BK = slice(NSK, NSK + 4)
EB_BQ = slice(NSK + 4, NSK + 4 + 8)
EB_C = NSK + 4 + 8
SL_EB = _seg(EB_C)
# small [1, x] vectors ride row 0 of their blob columns
SM_EV = _seg(VROW)              # 520
SM_ONES = _seg(W)               # 512
BLOB_C = _off


def build_nc():
    nc = bacc.Bacc("TRN2", target_bir_lowering=False, debug=False,
                   num_devices=1)

    blob = nc.dram_tensor("blob", [P, BLOB_C], BF, kind="ExternalInput")
    outp = nc.dram_tensor("outp", [W, D], BF, kind="ExternalOutput")

    Exp = mybir.ActivationFunctionType.Exp

    with tile.TileContext(nc) as tc:
        with (
            tc.tile_pool(name="persist", bufs=1) as pp,
            tc.tile_pool(name="psum", bufs=1, space="PSUM") as ps,
            tc.tile_pool(name="p1", bufs=1) as p1,
            tc.tile_pool(name="rope_tmp", bufs=1) as rt,
            tc.tile_pool(name="exp_pool", bufs=4) as epool,
            tc.tile_pool(name="norm_pool", bufs=1) as npo,
            tc.tile_pool(name="out_pool", bufs=2) as op,
        ):
            # ---- persistent tiles ----
            xbig = pp.tile([P, NK * S], BF, tag="xbig")
            xts = [xbig[:, k * S:(k + 1) * S] for k in range(NK)]
            kts = [pp.tile([P, S], BF, tag=f"kT{t}", name=f"kT{t}")
                   for t in range(4)]
            qts = [pp.tile([P, W], BF, tag=f"qT{p}", name=f"qT{p}")
                   for p in range(8)]
            vv = pp.tile([P, NSK * VROW], BF, tag="vv")
            aos = [pp.tile([P, W], BF, tag=f"ao{p}", name=f"ao{p}")
                   for p in range(8)]
            msk = pp.tile([P, 4 * W], BF, tag="msk")
            ebf_sb = pp.tile([P, EB_C], BF, tag="ebf")
            eb_sb = pp.tile([P, EB_C], FP, tag="eb")
            ones_sb = pp.tile([1, W], BF, tag="ones")

            # ---- phase-1 tiles ----
            wqbig = p1.tile([P, NK * H_Q * DK], BF, tag="wqbig")
            wqs = [wqbig[:, k * H_Q * DK:(k + 1) * H_Q * DK]
                   for k in range(NK)]
            wkbig = p1.tile([P, NK * H_KV * DK], BF, tag="wkbig")
            wks = [wkbig[:, k * H_KV * DK:(k + 1) * H_KV * DK]
                   for k in range(NK)]
            wvbig = p1.tile([P, NK * VROW], BF, tag="wvbig")
            wvs = [wvbig[:, k * VROW:(k + 1) * VROW] for k in range(NK)]
            cos_sb = p1.tile([P, S], BF, tag="cos")
            sin_sb = p1.tile([P, S], BF, tag="sin")
            ev_sb = p1.tile([1, VROW], BF, tag="ev")

            # ---- input DMAs, prefetch-ordered by first use ----
            # SP queue: pieces the first PE ops wait on; Pool queue (SWDGE):
            # big/late pieces.
            xb3d = blob[:, SL_XT].rearrange("p (k s) -> p k s", k=NK)
            xb3s = xbig[:].rearrange("p (k s) -> p k s", k=NK)
            wq3d = blob[:, SL_WQ].rearrange("p (k c) -> p k c", k=NK)
            wq3s = wqbig[:].rearrange("p (k c) -> p k c", k=NK)
            wk3d = blob[:, SL_WK].rearrange("p (k c) -> p k c", k=NK)
            wk3s = wkbig[:].rearrange("p (k c) -> p k c", k=NK)
            # window columns of x (slots 0-3 of every k-chunk) first
            nc.sync.dma_start(xb3s[:, :, 0:W], xb3d[:, :, 0:W])
            # q weights for pairs 0/1, then kT tile 0's weight columns
            nc.sync.dma_start(wq3s[:, :, 0:256], wq3d[:, :, 0:256])
            nc.sync.dma_start(cos_sb[:, 0:W], blob[:, SL_COS][:, 0:W])
            nc.sync.dma_start(sin_sb[:, 0:W], blob[:, SL_SIN][:, 0:W])
            nc.sync.dma_start(wk3s[:, :, 0:P], wk3d[:, :, 0:P])
            nc.sync.dma_start(wk3s[:, :, P:4 * P], wk3d[:, :, P:4 * P])
            nc.sync.dma_start(ebf_sb[:], blob[:, SL_EB])
            nc.vector.tensor_copy(eb_sb[:], ebf_sb[:])
            nc.sync.dma_start(ones_sb[:], blob[0:1, SM_ONES])
            nc.sync.dma_start(ev_sb[:], blob[0:1, SM_EV])
            nc.sync.dma_start(wvbig[:], blob[:, SL_WV])
            nc.sync.dma_start(msk[:], blob[:, SL_MSK])
            nc.gpsimd.dma_start(xb3s[:, :, W:2 * W], xb3d[:, :, W:2 * W])
            nc.gpsimd.dma_start(cos_sb[:, W:S], blob[:, SL_COS][:, W:S])
            nc.gpsimd.dma_start(sin_sb[:, W:S], blob[:, SL_SIN][:, W:S])
            nc.gpsimd.dma_start(xb3s[:, :, 2 * W:S], xb3d[:, :, 2 * W:S])
            nc.gpsimd.dma_start(wq3s[:, :, 256:1024], wq3d[:, :, 256:1024])

            Add, Mult = mybir.AluOpType.add, mybir.AluOpType.mult

            def rope_block(srcs, c_ap, s_ap, bias, dst, ncols):
                """RoPE a [128, ncols] projection (+ per-partition bias b):
                out = rope((x + b)). Partner rows sit +-16 partitions away
                (within one 32-row quadrant); the partner fetch is a DVE
                stream_shuffle. The sin table is pre-negated on x2 rows so
                the combine is one add: out = y*cos + shuffle16(y*sin')."""
                t_sb = rt.tile([P, 1024], BF, tag="ropeT", name="ropeT",
                               bufs=2)
                s_sb = rt.tile([P, 1024], BF, tag="ropeS", name="ropeS",
                               bufs=2)
                ss = rt.tile([P, 1024], BF, tag="ropeSS", name="ropeSS",
                             bufs=2)
                for ap, co, cw in srcs:
                    nc.vector.scalar_tensor_tensor(
                        t_sb[:, co:co + cw], ap, bias, c_ap[:, co:co + cw],
                        op0=Add, op1=Mult)
                    nc.vector.scalar_tensor_tensor(
                        s_sb[:, co:co + cw], ap, bias, s_ap[:, co:co + cw],
                        op0=Add, op1=Mult)
                nc.vector.stream_shuffle(ss[:, 0:ncols], s_sb[:, 0:ncols],
                                         SWAP16)
                nc.vector.tensor_add(dst, t_sb[:, 0:ncols], ss[:, 0:ncols])

            def emit_k(t, halves=(0, 1)):
                """K projection + RoPE for kT tile t = [kv_t | kv_{t+4}]."""
                for half in halves:       # 1024 seq cols per rope call
                    ho = half * 1024
                    srcs = []
                    for n in range(2):
                        pk = ps.tile([P, 512], FP, tag="B", name="pk",
                                     bufs=2, padded_shape=[P, 512])
                        for k in range(NK):
                            nc.tensor.matmul(
                                pk[:], wks[k][:, t * P:(t + 1) * P],
                                xts[k][:, ho + n * 512:ho + (n + 1) * 512],
                                start=(k == 0), stop=(k == NK - 1))
                        srcs.append((pk[:], n * 512, 512))
                    rope_block(srcs, cos_sb[:, ho:ho + 1024],
                               sin_sb[:, ho:ho + 1024],
                               eb_sb[:, NSK + t:NSK + t + 1],
                               kts[t][:, ho:ho + 1024], 1024)

            def emit_q1(p):
                """Q projection + RoPE for qT tile p (window = x slots 0-3)."""
                pq = ps.tile([P, 512], FP, tag="B", name="pq", bufs=2,
                             padded_shape=[P, 512])
                for k in range(NK):
                    nc.tensor.matmul(pq[:],
                                     wqs[k][:, p * P:(p + 1) * P],
                                     xts[k][:, 0:W], start=(k == 0),
                                     stop=(k == NK - 1))
                rope_block([(pq[:], 0, 512)], cos_sb[:, 0:W], sin_sb[:, 0:W],
                           eb_sb[:, NSK + 4 + p:NSK + 4 + p + 1],
                           qts[p][:], 512)

            def emit_v(i):
                """V projection for sk tile i (natural layout)."""
                for hh in range(2):       # 260 cols per half (4 heads)
                    col = slice(hh * 260, (hh + 1) * 260)
                    pv = ps.tile([P, 260], FP, tag="B", name="pv", bufs=2,
                                 padded_shape=[P, 512])
                    nc.tensor.matmul(pv[:], ones_sb[:, 0:P], ev_sb[:, col],
                                     start=True, stop=False)
                    for k in range(NK):
                        nc.tensor.matmul(pv[:],
                                         xts[k][:, i * P:(i + 1) * P],
                                         wvs[k][:, col], start=False,
                                         stop=(k == NK - 1))
                    nc.vector.tensor_copy(
                        vv[:, i * VROW + hh * 260:i * VROW + (hh + 1) * 260],
                        pv[:])

            def emit_pair(p, with_v=False, steps=None):
                """Attention for q heads (p, p+8). steps: {i: [callables]}
                emitted at the top of iteration i."""
                t = p // 2                # kT tile: kv p//2 | kv p//2+4
                av = [ps.tile([VC, 512], FP, tag="C", bufs=2,
                              padded_shape=[P, 512], name=f"av{h}")
                      for h in range(2)]
                for i in range(NSK):
                    for fn in (steps or {}).get(i, ()):
                        fn()
                    if with_v:
                        emit_v(i)
                    sc = ps.tile([P, 1024], FP, tag="A", name="sc", bufs=2)
                    for h in range(2):
                        r0 = h * 64
                        nc.tensor.matmul(
                            sc[:, h * 512:(h + 1) * 512],
                            kts[t][r0:r0 + 64, i * P:(i + 1) * P],
                            qts[p][r0:r0 + 64, :],
                            start=True, stop=True)
                    e = epool.tile([P, 1024], BF, tag="e", name="e", bufs=4)
                    nc.scalar.activation(e[:], sc[:], Exp, scale=SCALE,
                                         bias=eb_sb[:, i:i + 1])
                    if i < 4:             # diagonal tiles: pad|causal mask
                        e3 = e[:].rearrange("r (h w) -> r h w", h=2)
                        m3 = msk[:, i * W:(i + 1) * W].unsqueeze(1)
                        nc.vector.tensor_mul(e3, e3,
                                             m3.broadcast_to([P, 2, W]))
                    for h in range(2):
                        kv = t + h * 4    # kv head for q head p + h*8
                        vsl = slice(i * VROW + kv * VC,
                                    i * VROW + kv * VC + VC)
                        nc.tensor.matmul(av[h][:], vv[:, vsl],
                                         e[:, h * 512:(h + 1) * 512],
                                         start=(i == 0), stop=(i == NSK - 1),
                                         skip_group_check=True)
                # normalize: denominator rides row 64 of each av bank.
                # Evacuate PSUM to SBUF immediately (releases the av banks
                # for the next pair's accumulation), then the reciprocal
                # chain runs off the SBUF copy. The last pair has no
                # successor, so it skips the staging copies and reads PSUM
                # directly (shorter critical path into the output matmuls).
                avs = npo.tile([VC, 1024], FP, tag="avs", name="avs")
                rcs = npo.tile([1, 1024], FP, tag="rcs", name="rcs")
                bcs = npo.tile([64, 1024], FP, tag="bcs", name="bcs")
                st1 = npo.tile([64, W], BF, tag="st1", name="st1")
                if p < 7:
                    for h in range(2):
                        nc.vector.tensor_copy(avs[:, h * 512:(h + 1) * 512],
                                              av[h][:])
                    nc.sync.dma_start(rcs[0:1, :], avs[64:65, :])
                    num = [avs[0:64, 0:512], avs[0:64, 512:1024]]
                else:
                    for h in range(2):
                        nc.scalar.copy(avs[64:65, h * 512:(h + 1) * 512],
                                       av[h][64:65, :])
                    nc.sync.dma_start(rcs[0:1, :], avs[64:65, :])
                    num = [av[0][0:64, :], av[1][0:64, :]]
                nc.gpsimd.partition_broadcast(bcs[:], rcs[0:1, :])
                nc.vector.reciprocal(bcs[:], bcs[:])
                nc.vector.tensor_mul(aos[p][0:64, :], num[0], bcs[:, 0:512])
                nc.vector.tensor_mul(st1[:], num[1], bcs[:, 512:1024])
                nc.sync.dma_start(aos[p][64:128, :], st1[:])

            # ---- output projection helpers ----
            wo_t = []

            def po_partial(m, nh, po=None, tag="B"):
                if po is None:
                    po = ps.tile([P, 512], FP, tag=tag, name="po", bufs=2,
                                 padded_shape=[P, 512])[:]
                nsl = slice(nh * 512, (nh + 1) * 512)
                for k in range(NK - 1):
                    nc.tensor.matmul(
                        po, aos[k][:, m * P:(m + 1) * P],
                        wo_t[k][:, nsl], start=(k == 0), stop=False,
                        skip_group_check=True)
                return po

            def po_finish(m, nh, po):
                nsl = slice(nh * 512, (nh + 1) * 512)
                nc.tensor.matmul(
                    po, aos[NK - 1][:, m * P:(m + 1) * P],
                    wo_t[NK - 1][:, nsl], start=False, stop=True,
                    skip_group_check=True)
                osb = op.tile([P, 512], BF, tag="osb", name="osb")
                nc.scalar.copy(osb[:], po)
                nc.sync.dma_start(outp[m * P:(m + 1) * P, nsl], osb[:])

            # ---- interleaved emission ----
            # q0/q1 first: they only need xfirst+wq_first+cos/sin (all early
            # SP DMAs); the K rope waits on the slower Pool-queue cos/sin rest
            emit_q1(0)
            emit_q1(1)
            emit_k(0, halves=(0,))
            emit_pair(0, with_v=True,
                      steps={1: [lambda: emit_k(0, halves=(1,))],
                             4: [lambda: emit_q1(2)], 6: [lambda: emit_q1(3)],
                             8: [lambda: emit_q1(4)], 10: [lambda: emit_q1(5)],
                             12: [lambda: emit_q1(6)],
                             14: [lambda: emit_q1(7)]})
            # wo reuses the wq slots (same tag+shape, emitted after the last
            # Q-projection read so the WAR dependency orders correctly).
            wobig = p1.tile([P, NK * D], BF, tag="wqbig", name="wobig")
            wo_t.extend(wobig[:, k * D:(k + 1) * D] for k in range(NK))
            nc.gpsimd.dma_start(wobig[:], blob[:, SL_WO])
            emit_pair(1)
            emit_k(1)
            emit_pair(2)
            emit_pair(3)
            emit_k(2)
            emit_pair(4)
            emit_pair(5)
            emit_k(3)
            emit_pair(6)
            pos = {}
            emit_pair(7, steps={
                4: [lambda: pos.setdefault(0, po_partial(0, 0))],
                10: [lambda: pos.setdefault(1, po_partial(0, 1))]})
            # all partials (aos[0:7] only) before any finish (needs aos[7]),
            # so the PE's in-order queue isn't blocked behind the pair-7
            # normalize chain. m2/m3 share the two wide tag-A PSUM tiles.
            poc = [po_partial(1, nh, tag="C") for nh in range(2)]
            pow_ = [ps.tile([P, 1024], FP, tag="A", bufs=2,
                            name=f"poW{nh}") for nh in range(2)]
            poa = [po_partial(2, nh, po=pow_[nh][:, 0:512])
                   for nh in range(2)]
            pod = [po_partial(3, nh, po=pow_[nh][:, 512:1024])
                   for nh in range(2)]
            po_finish(0, 0, pos[0])
            po_finish(0, 1, pos[1])
            po_finish(1, 0, poc[0])
            po_finish(1, 1, poc[1])
            po_finish(2, 0, poa[0])
            po_finish(2, 1, poa[1])
            po_finish(3, 0, pod[0])
            po_finish(3, 1, pod[1])

    nc.compile()
    return nc


def _deint16_cols(Wm, h, dh):
    """Head h's dh columns, deinterleaved so the RoPE partner sits 16 rows
    away inside a 32-row quadrant: [x1 0:16 | x2 0:16 | x1 16:32 | x2 16:32]."""
    cols = Wm[:, h * dh:(h + 1) * dh]
    x1, x2 = cols[:, 0::2], cols[:, 1::2]
    return np.concatenate(
        [x1[:, 0:16], x2[:, 0:16], x1[:, 16:32], x2[:, 16:32]], axis=1)


def _deint16_vec(v, h, dh):
    seg = v[h * dh:(h + 1) * dh]
    x1, x2 = seg[0::2], seg[1::2]
    return np.concatenate([x1[0:16], x2[0:16], x1[16:32], x2[16:32]])


def _rope_tables(freqs_cos, freqs_sin):
    """[128, S] cos/sin tables matching the deint16 row layout; sin negated
    on x2 rows (partitions 16-31 mod 32) so the RoPE combine is one add."""
    f32 = np.float32
    ct = np.asarray(freqs_cos, f32).T   # [32, S]
    st = np.asarray(freqs_sin, f32).T
    rows_c = np.empty((P, ct.shape[1]), f32)
    rows_s = np.empty((P, ct.shape[1]), f32)
    for r in range(P):
        b, i = divmod(r, 32)
        pairidx = (b % 2) * 16 + (i % 16)
        rows_c[r] = ct[pairidx]
        rows_s[r] = st[pairidx] * (-1.0 if i >= 16 else 1.0)
    return rows_c, rows_s


def _prep_shared(Wq, bq, Wk, bk, Wv, bv, Wo, bo, freqs_cos, freqs_sin):
    """Host-side weight permutations shared by all cores."""
    f32 = np.float32
    bf = lambda a: np.ascontiguousarray(np.asarray(a, BF_NP))
    # q head order per m-tile p: [p | p+8]
    qorder = [h for p in range(8) for h in (p, p + 8)]
    # kv head order per kT tile t: [t | t+4]
    korder = [h for t in range(4) for h in (t, t + 4)]
    Wq_, Wk_, Wv_ = (np.asarray(Wq, f32), np.asarray(Wk, f32),
                     np.asarray(Wv, f32))
    wq_p = np.concatenate([_deint16_cols(Wq_, h, DK) for h in qorder], axis=1)
    wk_p = np.concatenate([_deint16_cols(Wk_, h, DK) for h in korder], axis=1)
    wk_p = wk_p.reshape(NK, P, H_KV * DK).transpose(1, 0, 2).reshape(
        P, NK * H_KV * DK)
    bq_p = np.concatenate([_deint16_vec(np.asarray(bq, f32), h, DK)
                           for h in qorder])
    bk_p = np.concatenate([_deint16_vec(np.asarray(bk, f32), h, DK)
                           for h in korder])
    zero = np.zeros((D, 1), f32)
    # kv heads 0-3 ride AV bank A with the ones column LAST ([V | 1]);
    # kv heads 4-7 ride bank B, same [V | 1] (denominator row 64 either way)
    wv_p = np.concatenate(
        [arr for h in range(H_KV)
         for arr in (Wv_[:, h * DV:(h + 1) * DV], zero)], axis=1)
    bv_ = np.asarray(bv, f32)
    ev = np.concatenate(
        [arr for h in range(H_KV)
         for arr in (bv_[h * DV:(h + 1) * DV], [1.0])]).astype(f32)
    Wo_ = np.asarray(Wo, f32).reshape(H_Q, DV, D)
    wo_p = Wo_[qorder].reshape(H_Q * DV, D)

    def dev(a):  # [NK*128, C] -> device big-tile layout [128, NK*C]
        return a.reshape(NK, P, -1).transpose(1, 0, 2).reshape(P, -1)

    wq_p, wv_p, wo_p = dev(wq_p), dev(wv_p), dev(wo_p)
    cosA, sinA = _rope_tables(freqs_cos, freqs_sin)
    smalls = np.zeros((P, SM_ONES.stop - SM_EV.start), f32)
    smalls[0, :VROW] = ev
    smalls[0, VROW:] = 1.0
    # per-partition projection biases, one column per kT / qT tile
    eb_bias = np.zeros((P, EB_C), f32)
    eb_bias[:, EB_BK] = bk_p.reshape(4, P).T
    eb_bias[:, EB_BQ] = bq_p.reshape(8, P).T
    return {
        "wq": bf(wq_p), "wk": bf(wk_p), "wv": bf(wv_p), "wo": bf(wo_p),
        "cosA": bf(cosA), "sinA": bf(sinA),
        "smalls": bf(smalls), "eb_bias": eb_bias,
        "bo": np.asarray(bo, f32),
    }


def _core_blob(shared, xTt_b, pad_b, j):
    """blob + ebias for the core with query window [j*512, (j+1)*512)."""
    f32 = np.float32
    o = j * W
    # key-tile permutation: diagonal tiles first
    order = list(range(4 * j, 4 * j + 4)) + list(range(0, 4 * j)) + \
        list(range(4 * j + 4, NSK))
    colperm = np.concatenate([np.arange(t * P, (t + 1) * P) for t in order])
    xTt_p = xTt_b[:, colperm]
    xT = np.ascontiguousarray(
        xTt_p.reshape(NK, P, S).transpose(1, 0, 2).reshape(P, NK * S))
    cosA = shared["cosA"][:, colperm]
    sinA = shared["sinA"][:, colperm]
    # diagonal mask, slots 0-3: vis[r, s, c] = pad[o+s*128+r] | (s*128+r <= c)
    r = np.arange(P)[:, None, None]
    s_ = np.arange(4)[None, :, None]
    c = np.arange(W)[None, None, :]
    kidx = o + s_ * P + r
    vis = pad_b[kidx.reshape(P, -1)].reshape(P, 4, 1) | (s_ * P + r <= c)
    maskM = vis.astype(f32).reshape(P, 4 * W)
    blob = np.empty((P, BLOB_C), BF_NP)
    blob[:, SL_XT] = xT
    blob[:, SL_WK] = shared["wk"]
    blob[:, SL_WQ] = shared["wq"]
    blob[:, SL_WV] = shared["wv"]
    blob[:, SL_COS] = cosA
    blob[:, SL_SIN] = sinA
    blob[:, SL_MSK] = maskM.astype(BF_NP)
    blob[:, SL_WO] = shared["wo"]
    # pad bias per slot: NEG on padded keys of tiles AFTER the window
    eb = shared["eb_bias"].copy()
    for slot in range(4, NSK):
        t_o = order[slot]
        if t_o > 4 * j + 3:
            eb[:, slot] = np.where(pad_b[t_o * P:(t_o + 1) * P], 0.0, NEG)
    blob[:, SL_EB] = eb.astype(BF_NP)
    blob[:, SL_EB.stop:BLOB_C] = shared["smalls"]
    return {"blob": blob}


_NC_CACHE = {}


def _get_nc():
    if "nc" not in _NC_CACHE:
        _NC_CACHE["nc"] = build_nc()
    return _NC_CACHE["nc"]


def _make_in_maps(x, Wq, bq, Wk, bk, Wv, bv, Wo, bo, freqs_cos, freqs_sin,
                  attention_mask):
    shared = _prep_shared(Wq, bq, Wk, bk, Wv, bv, Wo, bo, freqs_cos,
                          freqs_sin)
    xTts = [np.asarray(x[b], np.float32).T.astype(BF_NP) for b in range(B)]
    pads = [np.asarray(attention_mask[b]).astype(bool) for b in range(B)]
    in_maps = []
    for cix in range(N_CORES):
        b, j = cix // 4, cix % 4
        in_maps.append(_core_blob(shared, xTts[b], pads[b], j))
    return in_maps, shared


def kernel(x, Wq, bq, Wk, bk, Wv, bv, Wo, bo, freqs_cos, freqs_sin,
           attention_mask):
    nc = _get_nc()
    in_maps, shared = _make_in_maps(x, Wq, bq, Wk, bk, Wv, bv, Wo, bo,
                                    freqs_cos, freqs_sin, attention_mask)
    res = run_bass_kernel_spmd(nc, in_maps, core_ids=list(range(N_CORES)))
    out = np.empty((B, S, D), np.float32)
    for cix in range(N_CORES):
        b, j = cix // 4, cix % 4
        out[b, j * W:(j + 1) * W, :] = res.results[cix]["outp"].astype(
            np.float32)
    out += shared["bo"][None, None, :]
    return out


if __name__ == "__main__":
    rng = np.random.default_rng(0)
    ins = {
        "x": rng.standard_normal((B, S, D), dtype=np.float32),
        "Wq": rng.standard_normal((D, H_Q * DK), dtype=np.float32) * 0.02,
        "bq": np.zeros(H_Q * DK, np.float32),
        "Wk": rng.standard_normal((D, H_KV * DK), dtype=np.float32) * 0.02,
        "bk": np.zeros(H_KV * DK, np.float32),
        "Wv": rng.standard_normal((D, H_KV * DV), dtype=np.float32) * 0.02,
        "bv": np.zeros(H_KV * DV, np.float32),
        "Wo": rng.standard_normal((H_Q * DV, D), dtype=np.float32) * 0.02,
        "bo": np.zeros(D, np.float32),
        "freqs_cos": rng.standard_normal((S, DK // 2), dtype=np.float32),
        "freqs_sin": rng.standard_normal((S, DK // 2), dtype=np.float32),
        "attention_mask": rng.random((B, S)) < 0.9,
    }
    out = kernel(**ins)
    print("ran, out shape", out.shape, "finite:", np.isfinite(out).all())


# revision 5
# speedup vs baseline: 1.2234x; 1.2234x over previous
"""Trainium2 Bass kernel for GQA multi-head attention (RoPE + padding|causal mask).

Sequence-sharded, collective-free: 8 cores = 2 (batch) x 4 (query windows of
512 rows). Each core computes K/V for the full sequence, Q for its own window,
attention for all 16 q heads over its window, and its own [512, 1024] slice of
the output projection. No inter-core communication.

v2 redesign vs v1:
  * Per-core KEY-TILE PERMUTATION (host side): each core's 4 diagonal
    key tiles (the ones overlapping its query window) are moved to slots
    0-3; attention is permutation-invariant over keys. The explicit
    pad|causal mask multiply then only runs on slots 0-3. Slots 4-15 are
    either all-visible (tiles before the window: causal always holds) or
    pad-only (tiles after the window: causal never holds); the pad-only
    mask is folded into the exp as a per-partition bias of -30000 on
    padded keys (exp -> 0), which costs nothing (bias is already an
    activation operand).
  * Q-window inputs (xq/cosQ/sinQ) dropped: after permutation the window
    is always tiles 0-3, so Q reads fixed slices of xT/cosA.
  * RoPE partner swap via DVE stream_shuffle (partners laid out +-16
    partitions apart, inside one 32-row quadrant) instead of 4 SBUF->SBUF
    DMAs per rope block.
  * Softmax normalize reads AV PSUM directly (no staging copies); the two
    denominator rows are copied to SBUF by ScalarE, one partition
    broadcast + one reciprocal serve both heads.
  * All [128, C] bf16 inputs consolidated into one "blob" tensor, [1, C]
    vectors into "smalls", plus a tiny fp32 "ebias": 3 input args total
    (per-arg per-call dispatch cost is ~20us under the PJRT path).
  * Output in bf16 (halves per-call output staging).
"""

import sys

if "/opt/trn_rl_repo" not in sys.path:
    sys.path.insert(0, "/opt/trn_rl_repo")

import numpy as np
import ml_dtypes

BF_NP = ml_dtypes.bfloat16

import concourse.mybir as mybir
import concourse.tile as tile
from concourse import bacc
from concourse.bass_utils import run_bass_kernel_spmd

B, S, D = 2, 2048, 1024
H_Q, H_KV, DK, DV = 16, 8, 64, 64
N_CORES = 8
P = 128
W = 512          # query window per core
FP = mybir.dt.float32
BF = mybir.dt.bfloat16
SCALE = 1.0 / 8.0  # 1/sqrt(DK)
NK = D // P      # 8 k-tiles over the model dim
NSK = S // P     # 16 sk tiles
VC = DV + 1      # 65 = V cols + ones col per head
VROW = H_KV * VC  # 520 cols of vv per sk tile
NEG = -30000.0   # "minus infinity" for the pad bias (exp -> 0)

# stream_shuffle mask: rotate each 32-partition quadrant by 16 (RoPE partner)
SWAP16 = list(range(16, 32)) + list(range(16))

# ---- blob column layout (bf16, [128, BLOB_C]) ----
_off = 0
def _seg(n):
    global _off
    s = slice(_off, _off + n)
    _off += n
    return s
SL_XT = _seg(NK * S)            # 16384
SL_WK = _seg(NK * H_KV * DK)    # 4096
SL_WQ = _seg(NK * H_Q * DK)     # 8192
SL_WV = _seg(NK * VROW)         # 4160
SL_COS = _seg(S)                # 2048
SL_SIN = _seg(S)                # 2048
SL_WO = _seg(NK * D)            # 8192
# ebias block (bf16 in the blob, converted to fp32 on device):
# exp pad-bias per slot, bk per kT tile, bq per qT tile, then the pad
# column (1.0/0.0) for each of the 4 diagonal tiles (mask built on device)
EB_PAD = slice(0, NSK)
EB_BK = slice(NSK, NSK + 4)
EB_BQ = slice(NSK + 4, NSK + 4 + 8)
EB_PADC = slice(NSK + 12, NSK + 16)
EB_C = NSK + 16
SL_EB = _seg(EB_C)
# small [1, x] vectors ride row 0 of their blob columns
SM_EV = _seg(VROW)              # 520
SM_ONES = _seg(W)               # 512
BLOB_C = _off


def build_nc():
    nc = bacc.Bacc("TRN2", target_bir_lowering=False, debug=False,
                   num_devices=1)

    blob = nc.dram_tensor("blob", [P, BLOB_C], BF, kind="ExternalInput")
    outp = nc.dram_tensor("outp", [W, D], BF, kind="ExternalOutput")

    Exp = mybir.ActivationFunctionType.Exp

    with tile.TileContext(nc) as tc:
        with (
            tc.tile_pool(name="persist", bufs=1) as pp,
            tc.tile_pool(name="psum", bufs=1, space="PSUM") as ps,
            tc.tile_pool(name="p1", bufs=1) as p1,
            tc.tile_pool(name="rope_tmp", bufs=1) as rt,
            tc.tile_pool(name="exp_pool", bufs=4) as epool,
            tc.tile_pool(name="norm_pool", bufs=1) as npo,
            tc.tile_pool(name="out_pool", bufs=2) as op,
        ):
            # ---- persistent tiles ----
            xbig = pp.tile([P, NK * S], BF, tag="xbig")
            xts = [xbig[:, k * S:(k + 1) * S] for k in range(NK)]
            kts = [pp.tile([P, S], BF, tag=f"kT{t}", name=f"kT{t}")
                   for t in range(4)]
            qts = [pp.tile([P, W], BF, tag=f"qT{p}", name=f"qT{p}")
                   for p in range(8)]
            vv = pp.tile([P, NSK * VROW], BF, tag="vv")
            aos = [pp.tile([P, W], BF, tag=f"ao{p}", name=f"ao{p}")
                   for p in range(8)]
            msk = pp.tile([P, 4 * W], BF, tag="msk")
            ebf_sb = pp.tile([P, EB_C], BF, tag="ebf")
            eb_sb = pp.tile([P, EB_C], FP, tag="eb")
            ones_sb = pp.tile([1, W], BF, tag="ones")

            # ---- phase-1 tiles ----
            wqbig = p1.tile([P, NK * H_Q * DK], BF, tag="wqbig")
            wqs = [wqbig[:, k * H_Q * DK:(k + 1) * H_Q * DK]
                   for k in range(NK)]
            wkbig = p1.tile([P, NK * H_KV * DK], BF, tag="wkbig")
            wks = [wkbig[:, k * H_KV * DK:(k + 1) * H_KV * DK]
                   for k in range(NK)]
            wvbig = p1.tile([P, NK * VROW], BF, tag="wvbig")
            wvs = [wvbig[:, k * VROW:(k + 1) * VROW] for k in range(NK)]
            cos_sb = p1.tile([P, S], BF, tag="cos")
            sin_sb = p1.tile([P, S], BF, tag="sin")
            ev_sb = p1.tile([1, VROW], BF, tag="ev")

            # ---- input DMAs, prefetch-ordered by first use ----
            # SP queue: pieces the first PE ops wait on; Pool queue (SWDGE):
            # big/late pieces.
            xb3d = blob[:, SL_XT].rearrange("p (k s) -> p k s", k=NK)
            xb3s = xbig[:].rearrange("p (k s) -> p k s", k=NK)
            wq3d = blob[:, SL_WQ].rearrange("p (k c) -> p k c", k=NK)
            wq3s = wqbig[:].rearrange("p (k c) -> p k c", k=NK)
            wk3d = blob[:, SL_WK].rearrange("p (k c) -> p k c", k=NK)
            wk3s = wkbig[:].rearrange("p (k c) -> p k c", k=NK)
            # window columns of x (slots 0-3 of every k-chunk) first
            nc.sync.dma_start(xb3s[:, :, 0:W], xb3d[:, :, 0:W])
            # q weights for pairs 0/1, then kT tile 0's weight columns
            nc.sync.dma_start(wq3s[:, :, 0:256], wq3d[:, :, 0:256])
            nc.sync.dma_start(cos_sb[:, 0:W], blob[:, SL_COS][:, 0:W])
            nc.sync.dma_start(sin_sb[:, 0:W], blob[:, SL_SIN][:, 0:W])
            nc.sync.dma_start(wk3s[:, :, 0:P], wk3d[:, :, 0:P])
            nc.sync.dma_start(wk3s[:, :, P:4 * P], wk3d[:, :, P:4 * P])
            nc.sync.dma_start(ebf_sb[:], blob[:, SL_EB])
            nc.vector.tensor_copy(eb_sb[:], ebf_sb[:])
            nc.sync.dma_start(ones_sb[:], blob[0:1, SM_ONES])
            nc.sync.dma_start(ev_sb[:], blob[0:1, SM_EV])
            nc.sync.dma_start(wvbig[:], blob[:, SL_WV])
            nc.sync.dma_start(wq3s[:, :, 256:1024], wq3d[:, :, 256:1024])
            nc.gpsimd.dma_start(xb3s[:, :, W:2 * W], xb3d[:, :, W:2 * W])
            nc.gpsimd.dma_start(cos_sb[:, W:S], blob[:, SL_COS][:, W:S])
            nc.gpsimd.dma_start(sin_sb[:, W:S], blob[:, SL_SIN][:, W:S])
            # diagonal-tile masks built on device: start from all-visible,
            # zero the non-causal triangle, then OR the pad column back in
            # (max with the 0/1 pad bias column). iota = c - r - 128*s.
            IsGe = mybir.AluOpType.is_ge
            nc.gpsimd.memset(msk[:], 1.0)
            for s_ in range(4):
                msl = msk[:, s_ * W:(s_ + 1) * W]
                nc.gpsimd.affine_select(
                    msl, msl, pattern=[[1, W]], compare_op=IsGe, fill=0.0,
                    base=-(P * s_), channel_multiplier=-1)
                nc.gpsimd.tensor_scalar_max(
                    msl, msl, eb_sb[:, EB_PADC.start + s_:
                                    EB_PADC.start + s_ + 1])
            nc.gpsimd.dma_start(xb3s[:, :, 2 * W:S], xb3d[:, :, 2 * W:S])

            Add, Mult = mybir.AluOpType.add, mybir.AluOpType.mult

            def rope_block(srcs, c_ap, s_ap, bias, dst, ncols):
                """RoPE a [128, ncols] projection (+ per-partition bias b):
                out = rope((x + b)). Partner rows sit +-16 partitions away
                (within one 32-row quadrant); the partner fetch is a DVE
                stream_shuffle. The sin table is pre-negated on x2 rows so
                the combine is one add: out = y*cos + shuffle16(y*sin')."""
                t_sb = rt.tile([P, 1024], BF, tag="ropeT", name="ropeT",
                               bufs=2)
                s_sb = rt.tile([P, 1024], BF, tag="ropeS", name="ropeS",
                               bufs=2)
                ss = rt.tile([P, 1024], BF, tag="ropeSS", name="ropeSS",
                             bufs=2)
                for ap, co, cw in srcs:
                    nc.vector.scalar_tensor_tensor(
                        t_sb[:, co:co + cw], ap, bias, c_ap[:, co:co + cw],
                        op0=Add, op1=Mult)
                    nc.vector.scalar_tensor_tensor(
                        s_sb[:, co:co + cw], ap, bias, s_ap[:, co:co + cw],
                        op0=Add, op1=Mult)
                nc.vector.stream_shuffle(ss[:, 0:ncols], s_sb[:, 0:ncols],
                                         SWAP16)
                nc.vector.tensor_add(dst, t_sb[:, 0:ncols], ss[:, 0:ncols])

            def emit_k(t, halves=(0, 1)):
                """K projection + RoPE for kT tile t = [kv_t | kv_{t+4}]."""
                for half in halves:       # 1024 seq cols per rope call
                    ho = half * 1024
                    srcs = []
                    for n in range(2):
                        pk = ps.tile([P, 512], FP, tag="B", name="pk",
                                     bufs=2, padded_shape=[P, 512])
                        for k in range(NK):
                            nc.tensor.matmul(
                                pk[:], wks[k][:, t * P:(t + 1) * P],
                                xts[k][:, ho + n * 512:ho + (n + 1) * 512],
                                start=(k == 0), stop=(k == NK - 1))
                        srcs.append((pk[:], n * 512, 512))
                    rope_block(srcs, cos_sb[:, ho:ho + 1024],
                               sin_sb[:, ho:ho + 1024],
                               eb_sb[:, NSK + t:NSK + t + 1],
                               kts[t][:, ho:ho + 1024], 1024)

            def emit_q1(p):
                """Q projection + RoPE for qT tile p (window = x slots 0-3)."""
                pq = ps.tile([P, 512], FP, tag="B", name="pq", bufs=2,
                             padded_shape=[P, 512])
                for k in range(NK):
                    nc.tensor.matmul(pq[:],
                                     wqs[k][:, p * P:(p + 1) * P],
                                     xts[k][:, 0:W], start=(k == 0),
                                     stop=(k == NK - 1))
                rope_block([(pq[:], 0, 512)], cos_sb[:, 0:W], sin_sb[:, 0:W],
                           eb_sb[:, NSK + 4 + p:NSK + 4 + p + 1],
                           qts[p][:], 512)

            def emit_v(i):
                """V projection for sk tile i (natural layout)."""
                for hh in range(2):       # 260 cols per half (4 heads)
                    col = slice(hh * 260, (hh + 1) * 260)
                    pv = ps.tile([P, 260], FP, tag="B", name="pv", bufs=2,
                                 padded_shape=[P, 512])
                    nc.tensor.matmul(pv[:], ones_sb[:, 0:P], ev_sb[:, col],
                                     start=True, stop=False)
                    for k in range(NK):
                        nc.tensor.matmul(pv[:],
                                         xts[k][:, i * P:(i + 1) * P],
                                         wvs[k][:, col], start=False,
                                         stop=(k == NK - 1))
                    nc.vector.tensor_copy(
                        vv[:, i * VROW + hh * 260:i * VROW + (hh + 1) * 260],
                        pv[:])

            def emit_pair(p, with_v=False, steps=None):
                """Attention for q heads (p, p+8). steps: {i: [callables]}
                emitted at the top of iteration i."""
                t = p // 2                # kT tile: kv p//2 | kv p//2+4
                av = [ps.tile([VC, 512], FP, tag="C", bufs=2,
                              padded_shape=[P, 512], name=f"av{h}")
                      for h in range(2)]
                for i in range(NSK):
                    for fn in (steps or {}).get(i, ()):
                        fn()
                    if with_v:
                        emit_v(i)
                    sc = ps.tile([P, 1024], FP, tag="A", name="sc", bufs=2)
                    for h in range(2):
                        r0 = h * 64
                        nc.tensor.matmul(
                            sc[:, h * 512:(h + 1) * 512],
                            kts[t][r0:r0 + 64, i * P:(i + 1) * P],
                            qts[p][r0:r0 + 64, :],
                            start=True, stop=True)
                    e = epool.tile([P, 1024], BF, tag="e", name="e", bufs=4)
                    nc.scalar.activation(e[:], sc[:], Exp, scale=SCALE,
                                         bias=eb_sb[:, i:i + 1])
                    if i < 4:             # diagonal tiles: pad|causal mask
                        e3 = e[:].rearrange("r (h w) -> r h w", h=2)
                        m3 = msk[:, i * W:(i + 1) * W].unsqueeze(1)
                        nc.vector.tensor_mul(e3, e3,
                                             m3.broadcast_to([P, 2, W]))
                    for h in range(2):
                        kv = t + h * 4    # kv head for q head p + h*8
                        vsl = slice(i * VROW + kv * VC,
                                    i * VROW + kv * VC + VC)
                        nc.tensor.matmul(av[h][:], vv[:, vsl],
                                         e[:, h * 512:(h + 1) * 512],
                                         start=(i == 0), stop=(i == NSK - 1),
                                         skip_group_check=True)
                # normalize: denominator rides row 64 of each av bank.
                # Evacuate PSUM to SBUF immediately (releases the av banks
                # for the next pair's accumulation), then the reciprocal
                # chain runs off the SBUF copy. The last pair has no
                # successor, so it skips the staging copies and reads PSUM
                # directly (shorter critical path into the output matmuls).
                avs = npo.tile([VC, 1024], FP, tag="avs", name="avs")
                rcs = npo.tile([1, 1024], FP, tag="rcs", name="rcs")
                bcs = npo.tile([64, 1024], FP, tag="bcs", name="bcs")
                st1 = npo.tile([64, W], BF, tag="st1", name="st1")
                if p < 7:
                    for h in range(2):
                        nc.vector.tensor_copy(avs[:, h * 512:(h + 1) * 512],
                                              av[h][:])
                    nc.sync.dma_start(rcs[0:1, :], avs[64:65, :])
                    num = [avs[0:64, 0:512], avs[0:64, 512:1024]]
                else:
                    for h in range(2):
                        nc.scalar.copy(avs[64:65, h * 512:(h + 1) * 512],
                                       av[h][64:65, :])
                    nc.sync.dma_start(rcs[0:1, :], avs[64:65, :])
                    num = [av[0][0:64, :], av[1][0:64, :]]
                nc.gpsimd.partition_broadcast(bcs[:], rcs[0:1, :])
                nc.vector.reciprocal(bcs[:], bcs[:])
                nc.vector.tensor_mul(aos[p][0:64, :], num[0], bcs[:, 0:512])
                nc.vector.tensor_mul(st1[:], num[1], bcs[:, 512:1024])
                nc.sync.dma_start(aos[p][64:128, :], st1[:])

            # ---- output projection helpers ----
            wo_t = []

            def po_partial(m, nh, po=None, tag="B"):
                if po is None:
                    po = ps.tile([P, 512], FP, tag=tag, name="po", bufs=2,
                                 padded_shape=[P, 512])[:]
                nsl = slice(nh * 512, (nh + 1) * 512)
                for k in range(NK - 1):
                    nc.tensor.matmul(
                        po, aos[k][:, m * P:(m + 1) * P],
                        wo_t[k][:, nsl], start=(k == 0), stop=False,
                        skip_group_check=True)
                return po

            def po_finish(m, nh, po):
                nsl = slice(nh * 512, (nh + 1) * 512)
                nc.tensor.matmul(
                    po, aos[NK - 1][:, m * P:(m + 1) * P],
                    wo_t[NK - 1][:, nsl], start=False, stop=True,
                    skip_group_check=True)
                osb = op.tile([P, 512], BF, tag="osb", name="osb")
                nc.scalar.copy(osb[:], po)
                nc.sync.dma_start(outp[m * P:(m + 1) * P, nsl], osb[:])

            # ---- interleaved emission ----
            # q0/q1 first: they only need xfirst+wq_first+cos/sin (all early
            # SP DMAs); the K rope waits on the slower Pool-queue cos/sin rest
            emit_q1(0)
            emit_q1(1)
            emit_k(0, halves=(0,))
            emit_pair(0, with_v=True,
                      steps={1: [lambda: emit_k(0, halves=(1,))],
                             4: [lambda: emit_q1(2)], 6: [lambda: emit_q1(3)],
                             8: [lambda: emit_q1(4)], 10: [lambda: emit_q1(5)],
                             12: [lambda: emit_q1(6)],
                             14: [lambda: emit_q1(7)]})
            # wo reuses the wq slots (same tag+shape, emitted after the last
            # Q-projection read so the WAR dependency orders correctly).
            wobig = p1.tile([P, NK * D], BF, tag="wqbig", name="wobig")
            wo_t.extend(wobig[:, k * D:(k + 1) * D] for k in range(NK))
            nc.gpsimd.dma_start(wobig[:], blob[:, SL_WO])
            emit_pair(1)
            emit_k(1)
            emit_pair(2)
            emit_pair(3)
            emit_k(2)
            emit_pair(4)
            emit_pair(5)
            emit_k(3)
            emit_pair(6)
            pos = {}
            emit_pair(7, steps={
                4: [lambda: pos.setdefault(0, po_partial(0, 0))],
                10: [lambda: pos.setdefault(1, po_partial(0, 1))]})
            # all partials (aos[0:7] only) before any finish (needs aos[7]),
            # so the PE's in-order queue isn't blocked behind the pair-7
            # normalize chain. m2/m3 share the two wide tag-A PSUM tiles.
            poc = [po_partial(1, nh, tag="C") for nh in range(2)]
            pow_ = [ps.tile([P, 1024], FP, tag="A", bufs=2,
                            name=f"poW{nh}") for nh in range(2)]
            poa = [po_partial(2, nh, po=pow_[nh][:, 0:512])
                   for nh in range(2)]
            pod = [po_partial(3, nh, po=pow_[nh][:, 512:1024])
                   for nh in range(2)]
            po_finish(0, 0, pos[0])
            po_finish(0, 1, pos[1])
            po_finish(1, 0, poc[0])
            po_finish(1, 1, poc[1])
            po_finish(2, 0, poa[0])
            po_finish(2, 1, poa[1])
            po_finish(3, 0, pod[0])
            po_finish(3, 1, pod[1])

    nc.compile()
    return nc


def _deint16_cols(Wm, h, dh):
    """Head h's dh columns, deinterleaved so the RoPE partner sits 16 rows
    away inside a 32-row quadrant: [x1 0:16 | x2 0:16 | x1 16:32 | x2 16:32]."""
    cols = Wm[:, h * dh:(h + 1) * dh]
    x1, x2 = cols[:, 0::2], cols[:, 1::2]
    return np.concatenate(
        [x1[:, 0:16], x2[:, 0:16], x1[:, 16:32], x2[:, 16:32]], axis=1)


def _deint16_vec(v, h, dh):
    seg = v[h * dh:(h + 1) * dh]
    x1, x2 = seg[0::2], seg[1::2]
    return np.concatenate([x1[0:16], x2[0:16], x1[16:32], x2[16:32]])


def _rope_tables(freqs_cos, freqs_sin):
    """[128, S] cos/sin tables matching the deint16 row layout; sin negated
    on x2 rows (partitions 16-31 mod 32) so the RoPE combine is one add."""
    f32 = np.float32
    ct = np.asarray(freqs_cos, f32).T   # [32, S]
    st = np.asarray(freqs_sin, f32).T
    rows_c = np.empty((P, ct.shape[1]), f32)
    rows_s = np.empty((P, ct.shape[1]), f32)
    for r in range(P):
        b, i = divmod(r, 32)
        pairidx = (b % 2) * 16 + (i % 16)
        rows_c[r] = ct[pairidx]
        rows_s[r] = st[pairidx] * (-1.0 if i >= 16 else 1.0)
    return rows_c, rows_s


def _prep_shared(Wq, bq, Wk, bk, Wv, bv, Wo, bo, freqs_cos, freqs_sin):
    """Host-side weight permutations shared by all cores."""
    f32 = np.float32
    bf = lambda a: np.ascontiguousarray(np.asarray(a, BF_NP))
    # q head order per m-tile p: [p | p+8]
    qorder = [h for p in range(8) for h in (p, p + 8)]
    # kv head order per kT tile t: [t | t+4]
    korder = [h for t in range(4) for h in (t, t + 4)]
    Wq_, Wk_, Wv_ = (np.asarray(Wq, f32), np.asarray(Wk, f32),
                     np.asarray(Wv, f32))
    wq_p = np.concatenate([_deint16_cols(Wq_, h, DK) for h in qorder], axis=1)
    wk_p = np.concatenate([_deint16_cols(Wk_, h, DK) for h in korder], axis=1)
    wk_p = wk_p.reshape(NK, P, H_KV * DK).transpose(1, 0, 2).reshape(
        P, NK * H_KV * DK)
    bq_p = np.concatenate([_deint16_vec(np.asarray(bq, f32), h, DK)
                           for h in qorder])
    bk_p = np.concatenate([_deint16_vec(np.asarray(bk, f32), h, DK)
                           for h in korder])
    zero = np.zeros((D, 1), f32)
    # kv heads 0-3 ride AV bank A with the ones column LAST ([V | 1]);
    # kv heads 4-7 ride bank B, same [V | 1] (denominator row 64 either way)
    wv_p = np.concatenate(
        [arr for h in range(H_KV)
         for arr in (Wv_[:, h * DV:(h + 1) * DV], zero)], axis=1)
    bv_ = np.asarray(bv, f32)
    ev = np.concatenate(
        [arr for h in range(H_KV)
         for arr in (bv_[h * DV:(h + 1) * DV], [1.0])]).astype(f32)
    Wo_ = np.asarray(Wo, f32).reshape(H_Q, DV, D)
    wo_p = Wo_[qorder].reshape(H_Q * DV, D)

    def dev(a):  # [NK*128, C] -> device big-tile layout [128, NK*C]
        return a.reshape(NK, P, -1).transpose(1, 0, 2).reshape(P, -1)

    wq_p, wv_p, wo_p = dev(wq_p), dev(wv_p), dev(wo_p)
    cosA, sinA = _rope_tables(freqs_cos, freqs_sin)
    smalls = np.zeros((P, SM_ONES.stop - SM_EV.start), f32)
    smalls[0, :VROW] = ev
    smalls[0, VROW:] = 1.0
    # per-partition projection biases, one column per kT / qT tile
    eb_bias = np.zeros((P, EB_C), f32)
    eb_bias[:, EB_BK] = bk_p.reshape(4, P).T
    eb_bias[:, EB_BQ] = bq_p.reshape(8, P).T
    return {
        "wq": bf(wq_p), "wk": bf(wk_p), "wv": bf(wv_p), "wo": bf(wo_p),
        "cosA": bf(cosA), "sinA": bf(sinA),
        "smalls": bf(smalls), "eb_bias": eb_bias,
        "bo": np.asarray(bo, f32),
    }


def _core_blob(shared, xTt_b, pad_b, j):
    """blob + ebias for the core with query window [j*512, (j+1)*512)."""
    f32 = np.float32
    o = j * W
    # key-tile permutation: diagonal tiles first
    order = list(range(4 * j, 4 * j + 4)) + list(range(0, 4 * j)) + \
        list(range(4 * j + 4, NSK))
    colperm = np.concatenate([np.arange(t * P, (t + 1) * P) for t in order])
    xTt_p = xTt_b[:, colperm]
    xT = np.ascontiguousarray(
        xTt_p.reshape(NK, P, S).transpose(1, 0, 2).reshape(P, NK * S))
    cosA = shared["cosA"][:, colperm]
    sinA = shared["sinA"][:, colperm]
    blob = np.empty((P, BLOB_C), BF_NP)
    blob[:, SL_XT] = xT
    blob[:, SL_WK] = shared["wk"]
    blob[:, SL_WQ] = shared["wq"]
    blob[:, SL_WV] = shared["wv"]
    blob[:, SL_COS] = cosA
    blob[:, SL_SIN] = sinA
    blob[:, SL_WO] = shared["wo"]
    # pad bias per slot: NEG on padded keys of tiles AFTER the window
    eb = shared["eb_bias"].copy()
    for slot in range(4, NSK):
        t_o = order[slot]
        if t_o > 4 * j + 3:
            eb[:, slot] = np.where(pad_b[t_o * P:(t_o + 1) * P], 0.0, NEG)
    # pad columns (1.0/0.0) of the 4 diagonal tiles, for the on-device mask
    eb[:, EB_PADC] = pad_b[o:o + 4 * P].reshape(4, P).T.astype(f32)
    blob[:, SL_EB] = eb.astype(BF_NP)
    blob[:, SL_EB.stop:BLOB_C] = shared["smalls"]
    return {"blob": blob}


_NC_CACHE = {}


def _get_nc():
    if "nc" not in _NC_CACHE:
        _NC_CACHE["nc"] = build_nc()
    return _NC_CACHE["nc"]


def _make_in_maps(x, Wq, bq, Wk, bk, Wv, bv, Wo, bo, freqs_cos, freqs_sin,
                  attention_mask):
    shared = _prep_shared(Wq, bq, Wk, bk, Wv, bv, Wo, bo, freqs_cos,
                          freqs_sin)
    xTts = [np.asarray(x[b], np.float32).T.astype(BF_NP) for b in range(B)]
    pads = [np.asarray(attention_mask[b]).astype(bool) for b in range(B)]
    in_maps = []
    for cix in range(N_CORES):
        b, j = cix // 4, cix % 4
        in_maps.append(_core_blob(shared, xTts[b], pads[b], j))
    return in_maps, shared


def kernel(x, Wq, bq, Wk, bk, Wv, bv, Wo, bo, freqs_cos, freqs_sin,
           attention_mask):
    nc = _get_nc()
    in_maps, shared = _make_in_maps(x, Wq, bq, Wk, bk, Wv, bv, Wo, bo,
                                    freqs_cos, freqs_sin, attention_mask)
    res = run_bass_kernel_spmd(nc, in_maps, core_ids=list(range(N_CORES)))
    out = np.empty((B, S, D), np.float32)
    for cix in range(N_CORES):
        b, j = cix // 4, cix % 4
        out[b, j * W:(j + 1) * W, :] = res.results[cix]["outp"].astype(
            np.float32)
    out += shared["bo"][None, None, :]
    return out


if __name__ == "__main__":
    rng = np.random.default_rng(0)
    ins = {
        "x": rng.standard_normal((B, S, D), dtype=np.float32),
        "Wq": rng.standard_normal((D, H_Q * DK), dtype=np.float32) * 0.02,
        "bq": np.zeros(H_Q * DK, np.float32),
        "Wk": rng.standard_normal((D, H_KV * DK), dtype=np.float32) * 0.02,
        "bk": np.zeros(H_KV * DK, np.float32),
        "Wv": rng.standard_normal((D, H_KV * DV), dtype=np.float32) * 0.02,
        "bv": np.zeros(H_KV * DV, np.float32),
        "Wo": rng.standard_normal((H_Q * DV, D), dtype=np.float32) * 0.02,
        "bo": np.zeros(D, np.float32),
        "freqs_cos": rng.standard_normal((S, DK // 2), dtype=np.float32),
        "freqs_sin": rng.standard_normal((S, DK // 2), dtype=np.float32),
        "attention_mask": rng.random((B, S)) < 0.9,
    }
    out = kernel(**ins)
    print("ran, out shape", out.shape, "finite:", np.isfinite(out).all())
